# revision 45
# baseline (speedup 1.0000x reference)
"""Trainium2 Bass kernel for nn_DenTargetTransformerConv (GNN message passing).

Strategy (graph/data parallel, dst-owner sharding across 8 NeuronCores):
  - Nodes are partitioned by dst-id range; each core owns N/8 nodes and all
    edges whose dst falls in its range.  The halo exchange of src features is
    materialized host-side as a per-core edge-expanded bf16 table (one
    256-byte [q|v] row per edge slot, rows laid out in consumption order), so
    the device reads it with plain strided hardware-DGE DMAs at full
    bandwidth - no gathers.  Each run's table slice is fetched as four
    pieces alternating between the SP and Activation HWDGE queue pools so
    ~12 DMA queues stay busy across the three in-flight runs.  Runs are
    processed big-K first so their poorly-amortized work hides under the
    DMA-bound phase and the cheap small-K runs form the tail.
  - Per core, own nodes are sorted by in-degree and packed into groups of
    128 (SBUF partition dim).  Consecutive groups are merged into runs of
    R <= 7 groups sharing a padded slot count Kp (R*Kp <= 96); all 8 cores
    share one compiled program.
  - All edge-phase math is bf16 with (d,h)-minor layouts so every
    tensor_tensor hits the DVE 2x mode.  The two big reductions (score
    over d, aggregation over k) run on the tensor engine as identity-matmul
    slice accumulations into PSUM (errata-free 2.4 GHz, overlapped with
    DVE); the softmax denominator rides along in the same accumulation as 4
    extra columns, and the skip-side gate logit rides along in the k/skip
    linear as a 129th column.  Edge-softmax max-subtraction is elided
    (scores are O(+-5)).  Padded slots contribute exp(0)=1 to the
    denominator; a host-computed per-node correction (deg - Kp) fixes it.
  - Emission is software-pipelined one stage deep (run i's DMA/prod/score
    before run i-1's w/agg/copyout) so no engine head-of-line blocks on a
    cross-engine dependency.  The node phase (gate + LayerNorm + PReLU) is
    processed in chunks of ~2 runs woven between edge runs, so only the
    last chunk's short serial chain sits after the final run.  Everything
    scalar-engine-side (exp, tanh-sigmoid, square, prelu, copies) lives in
    the one `exp_and_others` activation table - zero table reloads; rsqrt
    for LayerNorm is a quake-style bit hack + 2 Newton steps on tiny
    [128, G] tiles on DVE.
"""

import numpy as np
import ml_dtypes

import concourse.bacc as bacc
import concourse.bass as bass
import concourse.tile as tile
from concourse import mybir
from concourse.bass_utils import run_bass_kernel_spmd

F32 = mybir.dt.float32
I32 = mybir.dt.int32
BF16 = mybir.dt.bfloat16
FP8 = mybir.dt.float8e4
BF = ml_dtypes.bfloat16
F8 = ml_dtypes.float8_e4m3
AX = mybir.AxisListType
ALU = mybir.AluOpType
ACTF = mybir.ActivationFunctionType

P = 128
NCORES = 8
HD = 64          # H * D
H, D = 4, 16
IN_F = 64

RMAX = 7         # groups per run  (R*68 <= 476 PSUM f32 bank)
RKMAX = 96       # slot-columns per run (SBUF tiles)
MAGIC = 0x5F3759DF


# ----------------------------------------------------------------- host prep

def _plan(q_src, v_src, feat, src, dst, ncores):
    n = feat.shape[0]
    npc = n // ncores
    ngrp = (npc + P - 1) // P
    grid = ngrp * P
    ndum = grid - npc

    # per-node [q | v] rows in (d, h)-minor order, bf16
    q2 = np.asarray(q_src, np.float32).reshape(n, H, D).transpose(0, 2, 1)
    v2 = np.asarray(v_src, np.float32).reshape(n, H, D).transpose(0, 2, 1)
    qv = np.concatenate([q2.reshape(n, HD), v2.reshape(n, HD)],
                        axis=1).astype(BF)          # [n, 128]

    src = np.asarray(src).astype(np.int64)
    dst = np.asarray(dst).astype(np.int64)
    order = np.argsort(dst, kind="stable")
    dst_s, src_s = dst[order], src[order]
    bounds = np.searchsorted(dst_s, np.arange(ncores + 1) * npc)

    cores = []
    gmax = np.zeros((ncores, ngrp), np.int64)
    gdegs = []
    for c in range(ncores):
        lo, hi = bounds[c], bounds[c + 1]
        dstL = dst_s[lo:hi] - c * npc          # ascending
        srcL = src_s[lo:hi]
        deg = np.bincount(dstL, minlength=npc)
        starts = np.concatenate([[0], np.cumsum(deg)])
        rank = np.arange(len(dstL)) - starts[dstL]
        perm = np.argsort(deg, kind="stable")  # ascending degree
        pos_of = np.empty(npc, np.int64)
        pos_of[perm] = ndum + np.arange(npc)
        gd = np.zeros(grid, np.int64)
        gd[ndum:] = deg[perm]
        gdeg = gd.reshape(ngrp, P)             # [g, p]
        gmax[c] = gdeg.max(1)
        gdegs.append(gdeg)
        cores.append(dict(dstL=dstL, srcL=srcL, rank=rank, perm=perm,
                          pos_of=pos_of))

    K = np.maximum(gmax.max(0), 1)             # shared per-group slot count

    # runs of consecutive groups, padded to the run max Kp
    runs = []       # (g0, g1, Kp)
    g = 0
    while g < ngrp:
        ge = g + 1
        while (ge < ngrp and ge - g < RMAX
               and (ge - g + 1) * K[ge] <= RKMAX
               and K[ge] <= K[g] * 1.3 + 2):
            ge += 1
        runs.append((g, ge, int(K[ge - 1])))
        g = ge
    # split the two smallest-K runs (processed last) so the final
    # serial chains after the DMA drains are half as long
    split = []
    for idx, (g0, g1, kp) in enumerate(runs):
        if idx < 2 and g1 - g0 >= 4:
            mid = (g0 + g1) // 2
            split += [(g0, mid, kp), (mid, g1, kp)]
        else:
            split.append((g0, g1, kp))
    runs = split
    Kpad = np.zeros(ngrp, np.int64)
    for (g0, g1, kp) in runs:
        Kpad[g0:g1] = kp
    assert Kpad.max() <= RKMAX

    colbase = np.concatenate([[0], np.cumsum(Kpad)]).astype(np.int64)
    totc = int(colbase[-1])

    # process runs big-K first: their small-R work hides under the
    # DMA-bound phase and the cheap small-K runs land in the tail
    porder = list(reversed(range(len(runs))))

    # per-core tab + denominator correction
    per_core = []
    for c in range(ncores):
        cd = cores[c]
        pos_e = cd["pos_of"][cd["dstL"]]       # grid position of each edge
        g_e = pos_e // P
        p_e = pos_e % P
        tab = np.zeros((totc * P, 2 * HD), BF)
        rows = colbase[g_e] * P + p_e * Kpad[g_e] + cd["rank"]
        tab[rows] = qv[cd["srcL"]]
        corr = (gdegs[c].T.astype(np.float32) -
                Kpad[None, :].astype(np.float32)) + 1e-9       # [P, ngrp]
        corr_w = np.repeat(corr, H, axis=1).astype(np.float32)  # [P, ngrp*H]
        per_core.append(dict(tab=tab, corr=corr_w))

    # featT with ones row, per core, grid-permuted: [IN_F+1, grid] bf16
    featTs = []
    feat = np.asarray(feat, np.float32)
    for c in range(ncores):
        ft = np.zeros((IN_F + 1, grid), np.float32)
        ft[IN_F, :] = 1.0
        perm = cores[c]["perm"]
        ft[:IN_F, ndum:] = feat[c * npc + perm].T
        featTs.append(ft.astype(BF))

    ident = np.eye(P, dtype=F8)

    return dict(n=n, npc=npc, ngrp=ngrp, grid=grid, ndum=ndum, K=K,
                Kpad=Kpad, colbase=colbase, totc=totc, runs=runs,
                porder=porder, ident=ident, cores=cores, per_core=per_core,
                featTs=featTs)


# standard (h,d) column index -> (d,h) position
def _dh_perm():
    cm = np.empty(HD, np.int64)
    for d in range(D):
        for h in range(H):
            cm[d * H + h] = h * D + d
    return cm            # newcol j takes oldcol cm[j]


# ------------------------------------------------------------- device build

def _build_nc(plan, ncores, trivial_affine=False):
    ngrp = plan["ngrp"]
    grid = plan["grid"]
    colbase = plan["colbase"]
    NG = ngrp
    runs = [plan["runs"][j] for j in plan["porder"]]   # processing order
    nruns = len(runs)
    # node-phase chunks over the processing order: pairs early, singles at
    # the end so the final serial chains are tiny
    chunks = []     # (run_lo, run_hi, g_lo, g_hi)
    i = 0
    while i < nruns:
        j = min(i + 2, nruns) if i < nruns - 3 else i + 1
        chunks.append((i, j, min(r[0] for r in runs[i:j]),
                       max(r[1] for r in runs[i:j])))
        i = j

    nc = bacc.Bacc("TRN2", target_bir_lowering=False, debug=False,
                   num_devices=ncores)

    featT_d = nc.dram_tensor("featT", [IN_F + 1, grid], BF16,
                             kind="ExternalInput").ap()
    wkws_d = nc.dram_tensor("wkws", [IN_F + 1, 132], BF16,
                            kind="ExternalInput").ap()
    tab_d = nc.dram_tensor("tab", [plan["totc"] * P, 2 * HD], BF16,
                           kind="ExternalInput").ap()
    corr_d = nc.dram_tensor("corr", [P, ngrp * H], F32,
                            kind="ExternalInput").ap()
    ident_d = nc.dram_tensor("ident", [P, P], FP8, kind="ExternalInput").ap()
    # bf16 params row: [wg2'(64) | gamma(64) | beta(64)]
    parb_d = nc.dram_tensor("parb", [1, 3 * HD], BF16,
                            kind="ExternalInput").ap()
    # f32 params row: [bgate/2, prelu_a, unused]
    parf_d = nc.dram_tensor("parf", [1, 3], F32, kind="ExternalInput").ap()
    out_d = nc.dram_tensor("out", [P, ngrp * HD], BF16,
                           kind="ExternalOutput").ap()

    with tile.TileContext(nc) as tc:
        with (
            tc.tile_pool(name="singles", bufs=1) as singles,
            tc.tile_pool(name="psL", bufs=2, space="PSUM") as psL,
            tc.tile_pool(name="psS", bufs=2, space="PSUM") as psS,
            tc.tile_pool(name="psA", bufs=2, space="PSUM") as psA,
            tc.tile_pool(name="qvp", bufs=3) as qvp,
            tc.tile_pool(name="prodp", bufs=2) as prodp,
            tc.tile_pool(name="wp", bufs=2) as wp,
            tc.tile_pool(name="nodep", bufs=2) as nodep,
        ):
            # ---- static loads (featT split 4-ways across both HWDGE pools)
            featT = singles.tile([IN_F + 1, grid], BF16)
            fq = grid // 4
            for fi in range(4):
                eng = nc.sync if fi % 2 == 0 else nc.scalar
                eng.dma_start(out=featT[:, fi * fq:(fi + 1) * fq],
                              in_=featT_d[:, fi * fq:(fi + 1) * fq])
            wkws = singles.tile([IN_F + 1, 132], BF16)
            nc.sync.dma_start(out=wkws[:], in_=wkws_d[:])
            ident = singles.tile([P, P], FP8)
            nc.scalar.dma_start(out=ident[:], in_=ident_d[:])
            corr_sb = singles.tile([P, ngrp * H], F32)
            nc.sync.dma_start(out=corr_sb[:], in_=corr_d[:])
            parb = singles.tile([P, 3 * HD], BF16)
            nc.gpsimd.dma_start(
                out=parb[:],
                in_=bass.AP(tensor=parb_d.tensor, offset=parb_d.offset,
                            ap=[[0, P], [1, 3 * HD]]))
            parf = singles.tile([P, 3], F32)
            nc.gpsimd.dma_start(
                out=parf[:],
                in_=bass.AP(tensor=parf_d.tensor, offset=parf_d.offset,
                            ap=[[0, P], [1, 3]]))
            bg2 = parf[:, 0:1]
            pa = parf[:, 1:2]

            # ---- per-node linears on PE: ks[:, g*128:...] = [k | skip],
            # plus the skip-side gate logit r1 as column 128.
            ks_sb = singles.tile([P, ngrp * 2 * HD], BF16)
            r1_sb = singles.tile([P, ngrp], F32)
            for q0 in reversed(range(0, ngrp, 3)):
                q1 = min(q0 + 3, ngrp)
                nq = q1 - q0
                pl = psL.tile([P, 3 * 132], F32, tag="pl")
                for g in range(q0, q1):
                    nc.tensor.matmul(out=pl[:, (g - q0) * 132:(g - q0 + 1) * 132],
                                     lhsT=featT[:, g * P:(g + 1) * P],
                                     rhs=wkws[:], start=True, stop=True)
                plv = pl[:, 0:1]
                nc.scalar.activation(
                    out=bass.AP(tensor=ks_sb[:].tensor,
                                offset=ks_sb[:].offset + q0 * 2 * HD,
                                ap=[ks_sb[:].ap[0], [2 * HD, nq], [1, 2 * HD]]),
                    in_=bass.AP(tensor=plv.tensor, offset=plv.offset,
                                ap=[plv.ap[0], [132, nq], [1, 2 * HD]]),
                    func=ACTF.Copy)
                nc.scalar.activation(
                    out=r1_sb[:, q0:q1],
                    in_=bass.AP(tensor=plv.tensor, offset=plv.offset + 128,
                                ap=[plv.ap[0], [132, nq], [1, 1]]),
                    func=ACTF.Copy)

            agg_sb = singles.tile([P, ngrp * HD], BF16)
            den_sb = singles.tile([P, ngrp * H], F32)
            ksv = ks_sb[:, 0:1]
            pb = parb[:, 0:1]

            # ------------------------------------------------ stage emitters
            qv_tiles = {}
            prod_tiles = {}
            ps_tiles = {}
            w_tiles = {}
            pa_tiles = {}

            def s0_dma(i):
                (g0, g1, K) = runs[i]
                R = g1 - g0
                RK = R * K
                r0 = int(colbase[g0]) * P
                qv_t = qvp.tile([P, RKMAX * 2 * HD], BF16, tag="qv")
                qv_tiles[i] = qv_t
                if R >= 2:
                    rh = R // 2
                    pieces = [(0, 64, 0, rh), (64, 64, 0, rh),
                              (0, 64, rh, R - rh), (64, 64, rh, R - rh)]
                else:
                    pieces = [(32 * q, 32, 0, 1) for q in range(4)]
                for pi, (p0, pn, ra, rn) in enumerate(pieces):
                    eng = nc.sync if pi % 2 == 0 else nc.scalar
                    in_ap = bass.AP(
                        tensor=tab_d.tensor,
                        offset=(tab_d.offset + r0 * 2 * HD
                                + p0 * K * 2 * HD + ra * P * K * 2 * HD),
                        ap=[[K * 2 * HD, pn], [P * K * 2 * HD, rn],
                            [1, K * 2 * HD]])
                    eng.dma_start(
                        out=qv_t[p0:p0 + pn,
                                 ra * K * 2 * HD:(ra + rn) * K * 2 * HD],
                        in_=in_ap)

            def s1_prod(i):
                (g0, g1, K) = runs[i]
                R = g1 - g0
                RK = R * K
                qv0 = qv_tiles[i][:, 0:1]
                prod = prodp.tile([P, RKMAX * HD], BF16, tag="prod")
                prod_tiles[i] = prod
                pv = prod[:, :RK * HD]
                p2 = bass.AP(tensor=pv.tensor, offset=pv.offset,
                             ap=[pv.ap[0], [HD, RK], [1, HD]])
                q2 = bass.AP(tensor=qv0.tensor, offset=qv0.offset,
                             ap=[qv0.ap[0], [2 * HD, RK], [1, HD]])
                kb = bass.AP(tensor=ksv.tensor, offset=ksv.offset + g0 * 2 * HD,
                             ap=[ksv.ap[0], [2 * HD, R], [0, K], [1, HD]])
                nc.vector.tensor_tensor(out=p2, in0=q2, in1=kb, op=ALU.mult)

            def s2_score(i):
                (g0, g1, K) = runs[i]
                RK = (g1 - g0) * K
                pv = prod_tiles[i][:, :RK * HD]
                ps = psS.tile([P, RKMAX * H], F32, tag="ps")
                ps_tiles[i] = ps
                sv = ps[:, :RK * H]
                for d in range(D):
                    rhs = bass.AP(tensor=pv.tensor, offset=pv.offset + d * H,
                                  ap=[pv.ap[0], [HD, RK], [1, H]])
                    nc.tensor.matmul(out=sv, lhsT=ident[:], rhs=rhs,
                                     start=(d == 0), stop=(d == D - 1))

            def s3_exp(i):
                (g0, g1, K) = runs[i]
                RK = (g1 - g0) * K
                sv = ps_tiles[i][:, :RK * H]
                w_t = wp.tile([P, RKMAX * 68], BF16, tag="w")
                w_tiles[i] = w_t
                wv = w_t[:, 0:1]
                exo = bass.AP(tensor=wv.tensor, offset=wv.offset + HD,
                              ap=[wv.ap[0], [68, RK], [1, H]])
                nc.scalar.activation(out=exo, in_=sv, func=ACTF.Exp,
                                     scale=0.25)

            def s4_w(i):
                (g0, g1, K) = runs[i]
                RK = (g1 - g0) * K
                qv0 = qv_tiles[i][:, 0:1]
                wv = w_tiles[i][:, 0:1]
                wo = bass.AP(tensor=wv.tensor, offset=wv.offset,
                             ap=[wv.ap[0], [68, RK], [1, HD]])
                vo = bass.AP(tensor=qv0.tensor, offset=qv0.offset + HD,
                             ap=[qv0.ap[0], [2 * HD, RK], [1, HD]])
                eb = bass.AP(tensor=wv.tensor, offset=wv.offset + HD,
                             ap=[wv.ap[0], [68, RK], [0, D], [1, H]])
                nc.vector.tensor_tensor(out=wo, in0=vo, in1=eb, op=ALU.mult)

            def s5_agg(i):
                (g0, g1, K) = runs[i]
                R = g1 - g0
                wv = w_tiles[i][:, 0:1]
                # pre-fold k-pairs on DVE for runs whose matmuls are small
                h = 0 if R >= 3 else 2
                cnt, step = K, 1
                for _ in range(h):
                    pairs = cnt // 2
                    if pairs == 0:
                        break
                    s68 = step * 68
                    lo = bass.AP(tensor=wv.tensor, offset=wv.offset,
                                 ap=[wv.ap[0], [68 * K, R], [2 * s68, pairs],
                                     [1, 68]])
                    hi = bass.AP(tensor=wv.tensor, offset=wv.offset + s68,
                                 ap=[wv.ap[0], [68 * K, R], [2 * s68, pairs],
                                     [1, 68]])
                    nc.vector.tensor_tensor(out=lo, in0=lo, in1=hi,
                                            op=ALU.add)
                    cnt = (cnt + 1) // 2
                    step *= 2
                pa_t = psA.tile([P, RMAX * 68], F32, tag="pa")
                pa_tiles[i] = pa_t
                av = pa_t[:, :R * 68]
                for k in range(cnt):
                    rhs = bass.AP(tensor=wv.tensor,
                                  offset=wv.offset + k * step * 68,
                                  ap=[wv.ap[0], [68 * K, R], [1, 68]])
                    nc.tensor.matmul(out=av, lhsT=ident[:], rhs=rhs,
                                     start=(k == 0), stop=(k == cnt - 1))

            def s6_copy(i):
                (g0, g1, K) = runs[i]
                R = g1 - g0
                av = pa_tiles[i][:, :R * 68]
                nc.scalar.activation(
                    out=bass.AP(tensor=agg_sb[:].tensor,
                                offset=agg_sb[:].offset + g0 * HD,
                                ap=[agg_sb[:].ap[0], [HD, R], [1, HD]]),
                    in_=bass.AP(tensor=av.tensor, offset=av.offset,
                                ap=[av.ap[0], [68, R], [1, HD]]),
                    func=ACTF.Copy)
                nc.scalar.activation(
                    out=bass.AP(tensor=den_sb[:].tensor,
                                offset=den_sb[:].offset + g0 * H,
                                ap=[den_sb[:].ap[0], [H, R], [1, H]]),
                    in_=bass.AP(tensor=av.tensor, offset=av.offset + HD,
                                ap=[av.ap[0], [68, R], [1, H]]),
                    func=ACTF.Copy)

            # -------------------------------------------- node-phase chunks
            # each chunk is emitted in 3 parts across consecutive iterations
            # so its cross-engine hops overlap a full edge run each.
            cstate = {}

            def node_a(ci):
                (_, _, ga, gb) = chunks[ci]
                G = gb - ga
                F = G * HD
                st = {}
                cstate[ci] = st
                dv = den_sb[:, ga * H:gb * H]
                nc.vector.tensor_tensor(out=dv, in0=dv,
                                        in1=corr_sb[:, ga * H:gb * H],
                                        op=ALU.add)
                nc.vector.reciprocal_approx_fast(out=dv, in_=dv)
                dinv = nodep.tile([P, RMAX * 2 * H], BF16, tag="dinv")
                nc.vector.tensor_scalar(out=dinv[:, :G * H], in0=dv,
                                        scalar1=1.0, scalar2=None,
                                        op0=ALU.mult)
                rst = nodep.tile([P, RMAX * 2 * HD], BF16, tag="rst")
                rv = rst[:, :F]
                r3 = bass.AP(tensor=rv.tensor, offset=rv.offset,
                             ap=[rv.ap[0], [HD, G], [1, HD]])
                a0 = agg_sb[:, 0:1]
                a3 = bass.AP(tensor=a0.tensor, offset=a0.offset + ga * HD,
                             ap=[a0.ap[0], [HD, G], [1, HD]])
                dq = dinv[:, 0:1]
                dinb = bass.AP(tensor=dq.tensor, offset=dq.offset,
                               ap=[dq.ap[0], [H, G], [0, D], [1, H]])
                nc.vector.tensor_tensor(out=r3, in0=a3, in1=dinb, op=ALU.mult)

                skipb = bass.AP(tensor=ksv.tensor,
                                offset=ksv.offset + ga * 2 * HD + HD,
                                ap=[ksv.ap[0], [2 * HD, G], [1, HD]])
                wg2b = bass.AP(tensor=pb.tensor, offset=pb.offset,
                               ap=[pb.ap[0], [0, G], [1, HD]])
                z = nodep.tile([P, RMAX * 2 * HD], BF16, tag="z")
                zv = z[:, :F]
                z3 = bass.AP(tensor=zv.tensor, offset=zv.offset,
                             ap=[zv.ap[0], [HD, G], [1, HD]])
                sc = nodep.tile([P, 8 * RMAX * 2], F32, tag="sc")
                r2 = sc[:, 0:G]
                gl = sc[:, G:2 * G]
                # gate logit: r2 = sum(rst*wg2'), gate = .5 + .5*tanh(...)
                nc.vector.tensor_tensor(out=z3, in0=r3, in1=wg2b, op=ALU.mult)
                nc.vector.tensor_reduce(out=r2, in_=z3, axis=AX.X, op=ALU.add)
                nc.vector.tensor_tensor(out=gl, in0=r2, in1=r1_sb[:, ga:gb],
                                        op=ALU.add)
                nc.scalar.activation(out=gl, in_=gl, func=ACTF.Tanh,
                                     scale=0.5, bias=bg2)
                gate = nodep.tile([P, RMAX * 2], BF16, tag="gate")
                nc.vector.tensor_scalar(out=gate[:, :G], in0=gl, scalar1=0.5,
                                        scalar2=0.5, op0=ALU.mult, op1=ALU.add)
                # dif = skip - rst (gpsimd), dif *= gate
                dif = nodep.tile([P, RMAX * 2 * HD], BF16, tag="dif")
                dv3 = bass.AP(tensor=dif[:].tensor, offset=dif[:].offset,
                              ap=[dif[:].ap[0], [HD, G], [1, HD]])
                nc.vector.tensor_tensor(out=dv3, in0=skipb, in1=r3,
                                        op=ALU.subtract)
                gq = gate[:, 0:1]
                gb_ = bass.AP(tensor=gq.tensor, offset=gq.offset,
                              ap=[gq.ap[0], [1, G], [0, HD]])
                nc.gpsimd.tensor_tensor(out=dv3, in0=dv3, in1=gb_,
                                        op=ALU.mult)
                st.update(G=G, F=F, rst=rst, rv=rv, r3=r3, z=z, zv=zv, z3=z3,
                          sc=sc, dif=dif)

            def node_b(ci):
                st = cstate[ci]
                G, F, rv, r3, zv, z3, sc = (st["G"], st["F"], st["rv"],
                                            st["r3"], st["zv"], st["z3"],
                                            st["sc"])
                vs = sc[:, 2 * G:3 * G]
                xh = sc[:, 3 * G:4 * G]
                t1 = sc[:, 4 * G:5 * G]
                mu = sc[:, 5 * G:6 * G]
                nc.vector.tensor_tensor(out=rv, in0=rv, in1=st["dif"][:, :F],
                                        op=ALU.add)
                # LayerNorm: mean, variance
                nc.vector.tensor_reduce(out=mu, in_=r3, axis=AX.X, op=ALU.add)
                mub = bass.AP(tensor=sc[:].tensor,
                              offset=sc[:].offset + 5 * G,
                              ap=[sc[:].ap[0], [1, G], [0, HD]])
                nc.vector.scalar_tensor_tensor(out=rv, in0=mub,
                                               scalar=-1.0 / HD, in1=rv,
                                               op0=ALU.mult, op1=ALU.add)
                nc.scalar.activation(out=zv, in_=rv, func=ACTF.Square)
                nc.vector.tensor_reduce(out=vs, in_=z3, axis=AX.X, op=ALU.add)
                nc.vector.tensor_scalar(out=vs, in0=vs, scalar1=1.0 / HD,
                                        scalar2=1e-5, op0=ALU.mult,
                                        op1=ALU.add)
                # quake rsqrt: y0 = bits(MAGIC - (i >> 1)); 2 Newton steps
                nc.vector.tensor_scalar(out=xh, in0=vs, scalar1=0.5,
                                        scalar2=None, op0=ALU.mult)
                vi = vs.bitcast(I32)
                nc.vector.tensor_scalar(out=vi, in0=vi, scalar1=1,
                                        scalar2=None,
                                        op0=ALU.logical_shift_right)
                nc.vector.tensor_scalar(out=vi, in0=vi, scalar1=-1,
                                        scalar2=MAGIC, op0=ALU.mult,
                                        op1=ALU.add)
                for _ in range(2):
                    nc.vector.tensor_tensor(out=t1, in0=vs, in1=vs,
                                            op=ALU.mult)
                    nc.vector.tensor_tensor(out=t1, in0=t1, in1=xh,
                                            op=ALU.mult)
                    nc.vector.tensor_scalar(out=t1, in0=t1, scalar1=-1.0,
                                            scalar2=1.5, op0=ALU.mult,
                                            op1=ALU.add)
                    nc.vector.tensor_tensor(out=vs, in0=vs, in1=t1,
                                            op=ALU.mult)
                isd = nodep.tile([P, RMAX * 2], BF16, tag="isd")
                nc.vector.tensor_scalar(out=isd[:, :G], in0=vs, scalar1=1.0,
                                        scalar2=None, op0=ALU.mult)
                st["isd"] = isd

            def node_c(ci):
                (_, _, ga, gb) = chunks[ci]
                st = cstate[ci]
                G, rv, r3 = st["G"], st["rv"], st["r3"]
                iq = st["isd"][:, 0:1]
                isb = bass.AP(tensor=iq.tensor, offset=iq.offset,
                              ap=[iq.ap[0], [1, G], [0, HD]])
                nc.gpsimd.tensor_tensor(out=rv, in0=rv, in1=isb, op=ALU.mult)
                if not trivial_affine:
                    gammab = bass.AP(tensor=pb.tensor, offset=pb.offset + HD,
                                     ap=[pb.ap[0], [0, G], [1, HD]])
                    betab = bass.AP(tensor=pb.tensor,
                                    offset=pb.offset + 2 * HD,
                                    ap=[pb.ap[0], [0, G], [1, HD]])
                    nc.gpsimd.tensor_tensor(out=r3, in0=r3, in1=gammab,
                                            op=ALU.mult)
                    nc.vector.tensor_tensor(out=r3, in0=r3, in1=betab,
                                            op=ALU.add)
                nc.scalar.activation(out=rv, in_=rv, func=ACTF.Prelu,
                                     alpha=pa)
                nc.gpsimd.dma_start(out=out_d[:, ga * HD:gb * HD], in_=rv)

            # ------------------------------------------------ emission loop
            part_a = {b: ci for ci, (a, b, _, _) in enumerate(chunks)}
            part_b = {b + 1: ci for ci, (a, b, _, _) in enumerate(chunks)}
            part_c = {b + 2: ci for ci, (a, b, _, _) in enumerate(chunks)}
            for i in range(nruns + 3):
                if i < nruns:
                    s0_dma(i)
                    s1_prod(i)
                    s2_score(i)
                    s3_exp(i)
                if 0 < i <= nruns:
                    s4_w(i - 1)
                    s5_agg(i - 1)
                    s6_copy(i - 1)
                if i in part_a:
                    node_a(part_a[i])
                if i in part_b:
                    node_b(part_b[i])
                if i in part_c:
                    node_c(part_c[i])

    nc.compile()
    return nc


# ------------------------------------------------------------------- driver

_CACHE = {}


def _get_nc(plan, ncores, trivial_affine):
    key = (tuple(plan["Kpad"].tolist()), plan["grid"], ncores, trivial_affine)
    if key not in _CACHE:
        _CACHE[key] = _build_nc(plan, ncores, trivial_affine)
    return _CACHE[key]


def _make_inmaps(plan, params, ncores):
    (Wk, bk, Wskip, bskip, Wgate, bgate, ln_gamma, ln_beta, prelu_a) = params
    cm = _dh_perm()
    wg = np.asarray(Wgate, np.float32).reshape(3 * HD)
    wg1 = wg[0:64] + wg[128:192]          # acts on skip
    wg2 = wg[64:128] - wg[128:192]        # acts on rst

    wk = np.concatenate([np.asarray(Wk, np.float32),
                         np.asarray(bk, np.float32).reshape(1, HD)])[:, cm]
    wsk_f = np.concatenate([np.asarray(Wskip, np.float32),
                            np.asarray(bskip, np.float32).reshape(1, HD)])
    wsk = wsk_f[:, cm]
    wkws = np.zeros((IN_F + 1, 132), np.float32)
    wkws[:, 0:HD] = wk
    wkws[:, HD:2 * HD] = wsk
    wkws[:, 128] = wsk_f @ wg1            # r1 column (skip-side gate logit)
    wkws = wkws.astype(BF)

    parb = np.zeros((1, 3 * HD), np.float32)
    parb[0, 0:HD] = wg2[cm]
    parb[0, HD:2 * HD] = np.asarray(ln_gamma, np.float32)[cm]
    parb[0, 2 * HD:3 * HD] = np.asarray(ln_beta, np.float32)[cm]
    parb = parb.astype(BF)
    parf = np.array([[np.float32(np.asarray(bgate).reshape(-1)[0]) * 0.5,
                      np.float32(np.asarray(prelu_a).reshape(-1)[0]),
                      0.0]], np.float32)

    in_maps = []
    for c in range(ncores):
        pc = plan["per_core"][c]
        m = dict(featT=plan["featTs"][c], tab=pc["tab"], corr=pc["corr"],
                 ident=plan["ident"], wkws=wkws, parb=parb, parf=parf)
        in_maps.append(m)
    return in_maps


def run(q_src, v_src, feat, src, dst, Wk, bk, Wskip, bskip, Wgate, bgate,
        ln_gamma, ln_beta, prelu_a, ncores=NCORES, trace=False):
    plan = _plan(q_src, v_src, feat, src, dst, ncores)
    trivial_affine = bool(
        np.all(np.asarray(ln_gamma, np.float32) == 1.0)
        and np.all(np.asarray(ln_beta, np.float32) == 0.0))
    nc = _get_nc(plan, ncores, trivial_affine)
    in_maps = _make_inmaps(
        plan, (Wk, bk, Wskip, bskip, Wgate, bgate, ln_gamma, ln_beta, prelu_a),
        ncores)
    res = run_bass_kernel_spmd(nc, in_maps, core_ids=list(range(ncores)),
                               trace=trace)
    n, npc, ngrp = plan["n"], plan["npc"], plan["ngrp"]
    out = np.empty((n, HD), np.float32)
    for c in range(ncores):
        r = np.asarray(res.results[c]["out"]).astype(np.float32)
        # [P, ngrp, D, H] -> [ngrp, P, H, D] -> [grid, HD]
        arr = r.reshape(P, ngrp, D, H).transpose(1, 0, 3, 2).reshape(-1, HD)
        out[c * npc + plan["cores"][c]["perm"]] = \
            arr[plan["ndum"]:plan["ndum"] + npc]
    return out, res, plan, in_maps, nc


def kernel(**inputs):
    out, _, _, _, _ = run(**inputs)
    return out


# revision 46
# speedup vs baseline: 1.0227x; 1.0227x over previous
"""Trainium2 Bass kernel for nn_DenTargetTransformerConv (GNN message passing).

Strategy (graph/data parallel, dst-owner sharding across 8 NeuronCores):
  - Nodes are partitioned by dst-id range; each core owns N/8 nodes and all
    edges whose dst falls in its range.  The halo exchange of src features is
    materialized host-side as a per-core edge-expanded bf16 table (one
    256-byte [q|v] row per edge slot, rows laid out in consumption order), so
    the device reads it with plain strided hardware-DGE DMAs at full
    bandwidth - no gathers.  Each run's table slice is fetched as four
    pieces alternating between the SP and Activation HWDGE queue pools so
    ~12 DMA queues stay busy across the three in-flight runs.  Runs are
    processed big-K first so their poorly-amortized work hides under the
    DMA-bound phase and the cheap small-K runs form the tail.
  - Per core, own nodes are sorted by in-degree and packed into groups of
    128 (SBUF partition dim).  Consecutive groups are merged into runs of
    R <= 7 groups sharing a padded slot count Kp (R*Kp <= 96); all 8 cores
    share one compiled program.
  - All edge-phase math is bf16 with (d,h)-minor layouts so every
    tensor_tensor hits the DVE 2x mode.  The two big reductions (score
    over d, aggregation over k) run on the tensor engine as identity-matmul
    slice accumulations into PSUM (errata-free 2.4 GHz, overlapped with
    DVE); the softmax denominator rides along in the same accumulation as 4
    extra columns, and the skip-side gate logit rides along in the k/skip
    linear as a 129th column.  Edge-softmax max-subtraction is elided
    (scores are O(+-5)).  Padded slots contribute exp(0)=1 to the
    denominator; a host-computed per-node correction (deg - Kp) fixes it.
  - Emission is software-pipelined one stage deep (run i's DMA/prod/score
    before run i-1's w/agg/copyout) so no engine head-of-line blocks on a
    cross-engine dependency.  The node phase (gate + LayerNorm + PReLU) is
    processed in chunks of ~2 runs woven between edge runs, so only the
    last chunk's short serial chain sits after the final run.  Everything
    scalar-engine-side (exp, tanh-sigmoid, square, prelu, copies) lives in
    the one `exp_and_others` activation table - zero table reloads; rsqrt
    for LayerNorm is a quake-style bit hack + 2 Newton steps on tiny
    [128, G] tiles on DVE.
"""

import numpy as np
import ml_dtypes

import concourse.bacc as bacc
import concourse.bass as bass
import concourse.tile as tile
from concourse import mybir
from concourse.bass_utils import run_bass_kernel_spmd

F32 = mybir.dt.float32
I32 = mybir.dt.int32
BF16 = mybir.dt.bfloat16
FP8 = mybir.dt.float8e4
BF = ml_dtypes.bfloat16
F8 = ml_dtypes.float8_e4m3
AX = mybir.AxisListType
ALU = mybir.AluOpType
ACTF = mybir.ActivationFunctionType

P = 128
NCORES = 8
HD = 64          # H * D
H, D = 4, 16
IN_F = 64

RMAX = 7         # groups per run  (R*68 <= 476 PSUM f32 bank)
RKMAX = 96       # slot-columns per run (SBUF tiles)
MAGIC = 0x5F3759DF


# ----------------------------------------------------------------- host prep

def _plan(q_src, v_src, feat, src, dst, ncores):
    n = feat.shape[0]
    npc = n // ncores
    ngrp = (npc + P - 1) // P
    grid = ngrp * P
    ndum = grid - npc

    # per-node [q | v] rows in (d, h)-minor order, bf16
    q2 = np.asarray(q_src, np.float32).reshape(n, H, D).transpose(0, 2, 1)
    v2 = np.asarray(v_src, np.float32).reshape(n, H, D).transpose(0, 2, 1)
    qv = np.concatenate([q2.reshape(n, HD), v2.reshape(n, HD)],
                        axis=1).astype(BF)          # [n, 128]

    src = np.asarray(src).astype(np.int64)
    dst = np.asarray(dst).astype(np.int64)
    order = np.argsort(dst, kind="stable")
    dst_s, src_s = dst[order], src[order]
    bounds = np.searchsorted(dst_s, np.arange(ncores + 1) * npc)

    cores = []
    gmax = np.zeros((ncores, ngrp), np.int64)
    gdegs = []
    for c in range(ncores):
        lo, hi = bounds[c], bounds[c + 1]
        dstL = dst_s[lo:hi] - c * npc          # ascending
        srcL = src_s[lo:hi]
        deg = np.bincount(dstL, minlength=npc)
        starts = np.concatenate([[0], np.cumsum(deg)])
        rank = np.arange(len(dstL)) - starts[dstL]
        perm = np.argsort(deg, kind="stable")  # ascending degree
        pos_of = np.empty(npc, np.int64)
        pos_of[perm] = ndum + np.arange(npc)
        gd = np.zeros(grid, np.int64)
        gd[ndum:] = deg[perm]
        gdeg = gd.reshape(ngrp, P)             # [g, p]
        gmax[c] = gdeg.max(1)
        gdegs.append(gdeg)
        cores.append(dict(dstL=dstL, srcL=srcL, rank=rank, perm=perm,
                          pos_of=pos_of))

    K = np.maximum(gmax.max(0), 1)             # shared per-group slot count

    # runs of consecutive groups, padded to the run max Kp
    runs = []       # (g0, g1, Kp)
    g = 0
    while g < ngrp:
        ge = g + 1
        while (ge < ngrp and ge - g < RMAX
               and (ge - g + 1) * K[ge] <= RKMAX
               and K[ge] <= K[g] * 1.3 + 2):
            ge += 1
        runs.append((g, ge, int(K[ge - 1])))
        g = ge
    Kpad = np.zeros(ngrp, np.int64)
    for (g0, g1, kp) in runs:
        Kpad[g0:g1] = kp
    assert Kpad.max() <= RKMAX

    colbase = np.concatenate([[0], np.cumsum(Kpad)]).astype(np.int64)
    totc = int(colbase[-1])

    # process runs big-K first: their small-R work hides under the
    # DMA-bound phase and the cheap small-K runs land in the tail
    porder = list(reversed(range(len(runs))))

    # per-core tab + denominator correction
    per_core = []
    for c in range(ncores):
        cd = cores[c]
        pos_e = cd["pos_of"][cd["dstL"]]       # grid position of each edge
        g_e = pos_e // P
        p_e = pos_e % P
        tab = np.zeros((totc * P, 2 * HD), BF)
        rows = colbase[g_e] * P + p_e * Kpad[g_e] + cd["rank"]
        tab[rows] = qv[cd["srcL"]]
        corr = (gdegs[c].T.astype(np.float32) -
                Kpad[None, :].astype(np.float32)) + 1e-9       # [P, ngrp]
        corr_w = np.repeat(corr, H, axis=1).astype(np.float32)  # [P, ngrp*H]
        per_core.append(dict(tab=tab, corr=corr_w))

    # featT with ones row, per core, grid-permuted: [IN_F+1, grid] bf16
    featTs = []
    feat = np.asarray(feat, np.float32)
    for c in range(ncores):
        ft = np.zeros((IN_F + 1, grid), np.float32)
        ft[IN_F, :] = 1.0
        perm = cores[c]["perm"]
        ft[:IN_F, ndum:] = feat[c * npc + perm].T
        featTs.append(ft.astype(BF))

    ident = np.eye(P, dtype=F8)

    return dict(n=n, npc=npc, ngrp=ngrp, grid=grid, ndum=ndum, K=K,
                Kpad=Kpad, colbase=colbase, totc=totc, runs=runs,
                porder=porder, ident=ident, cores=cores, per_core=per_core,
                featTs=featTs)


# standard (h,d) column index -> (d,h) position
def _dh_perm():
    cm = np.empty(HD, np.int64)
    for d in range(D):
        for h in range(H):
            cm[d * H + h] = h * D + d
    return cm            # newcol j takes oldcol cm[j]


# ------------------------------------------------------------- device build

def _build_nc(plan, ncores, trivial_affine=False):
    ngrp = plan["ngrp"]
    grid = plan["grid"]
    colbase = plan["colbase"]
    NG = ngrp
    runs = [plan["runs"][j] for j in plan["porder"]]   # processing order
    nruns = len(runs)
    # node-phase chunks over the processing order: pairs early, singles at
    # the end so the final serial chains are tiny
    chunks = []     # (run_lo, run_hi, g_lo, g_hi)
    i = 0
    while i < nruns:
        j = min(i + 2, nruns) if i < nruns - 3 else i + 1
        chunks.append((i, j, min(r[0] for r in runs[i:j]),
                       max(r[1] for r in runs[i:j])))
        i = j

    nc = bacc.Bacc("TRN2", target_bir_lowering=False, debug=False,
                   num_devices=ncores)

    featT_d = nc.dram_tensor("featT", [IN_F + 1, grid], BF16,
                             kind="ExternalInput").ap()
    wkws_d = nc.dram_tensor("wkws", [IN_F + 1, 132], BF16,
                            kind="ExternalInput").ap()
    tab_d = nc.dram_tensor("tab", [plan["totc"] * P, 2 * HD], BF16,
                           kind="ExternalInput").ap()
    corr_d = nc.dram_tensor("corr", [P, ngrp * H], F32,
                            kind="ExternalInput").ap()
    ident_d = nc.dram_tensor("ident", [P, P], FP8, kind="ExternalInput").ap()
    # bf16 params row: [wg2'(64) | gamma(64) | beta(64)]
    parb_d = nc.dram_tensor("parb", [1, 3 * HD], BF16,
                            kind="ExternalInput").ap()
    # f32 params row: [bgate/2, prelu_a, unused]
    parf_d = nc.dram_tensor("parf", [1, 3], F32, kind="ExternalInput").ap()
    out_d = nc.dram_tensor("out", [P, ngrp * HD], BF16,
                           kind="ExternalOutput").ap()

    with tile.TileContext(nc) as tc:
        with (
            tc.tile_pool(name="singles", bufs=1) as singles,
            tc.tile_pool(name="psL", bufs=2, space="PSUM") as psL,
            tc.tile_pool(name="psS", bufs=2, space="PSUM") as psS,
            tc.tile_pool(name="psA", bufs=2, space="PSUM") as psA,
            tc.tile_pool(name="qvp", bufs=3) as qvp,
            tc.tile_pool(name="prodp", bufs=2) as prodp,
            tc.tile_pool(name="wp", bufs=2) as wp,
            tc.tile_pool(name="nodep", bufs=2) as nodep,
        ):
            # ---- static loads (featT split 4-ways across both HWDGE pools)
            featT = singles.tile([IN_F + 1, grid], BF16)
            fq = grid // 4
            for fi in range(4):
                eng = nc.sync if fi % 2 == 0 else nc.scalar
                eng.dma_start(out=featT[:, fi * fq:(fi + 1) * fq],
                              in_=featT_d[:, fi * fq:(fi + 1) * fq])
            wkws = singles.tile([IN_F + 1, 132], BF16)
            nc.sync.dma_start(out=wkws[:], in_=wkws_d[:])
            ident = singles.tile([P, P], FP8)
            nc.scalar.dma_start(out=ident[:], in_=ident_d[:])
            corr_sb = singles.tile([P, ngrp * H], F32)
            nc.sync.dma_start(out=corr_sb[:], in_=corr_d[:])
            parb = singles.tile([P, 3 * HD], BF16)
            nc.gpsimd.dma_start(
                out=parb[:],
                in_=bass.AP(tensor=parb_d.tensor, offset=parb_d.offset,
                            ap=[[0, P], [1, 3 * HD]]))
            parf = singles.tile([P, 3], F32)
            nc.gpsimd.dma_start(
                out=parf[:],
                in_=bass.AP(tensor=parf_d.tensor, offset=parf_d.offset,
                            ap=[[0, P], [1, 3]]))
            bg2 = parf[:, 0:1]
            pa = parf[:, 1:2]

            # ---- per-node linears on PE: ks[:, g*128:...] = [k | skip],
            # plus the skip-side gate logit r1 as column 128.
            ks_sb = singles.tile([P, ngrp * 2 * HD], BF16)
            r1_sb = singles.tile([P, ngrp], F32)
            for q0 in reversed(range(0, ngrp, 3)):
                q1 = min(q0 + 3, ngrp)
                nq = q1 - q0
                pl = psL.tile([P, 3 * 132], F32, tag="pl")
                for g in range(q0, q1):
                    nc.tensor.matmul(out=pl[:, (g - q0) * 132:(g - q0 + 1) * 132],
                                     lhsT=featT[:, g * P:(g + 1) * P],
                                     rhs=wkws[:], start=True, stop=True)
                plv = pl[:, 0:1]
                nc.scalar.activation(
                    out=bass.AP(tensor=ks_sb[:].tensor,
                                offset=ks_sb[:].offset + q0 * 2 * HD,
                                ap=[ks_sb[:].ap[0], [2 * HD, nq], [1, 2 * HD]]),
                    in_=bass.AP(tensor=plv.tensor, offset=plv.offset,
                                ap=[plv.ap[0], [132, nq], [1, 2 * HD]]),
                    func=ACTF.Copy)
                nc.scalar.activation(
                    out=r1_sb[:, q0:q1],
                    in_=bass.AP(tensor=plv.tensor, offset=plv.offset + 128,
                                ap=[plv.ap[0], [132, nq], [1, 1]]),
                    func=ACTF.Copy)

            agg_sb = singles.tile([P, ngrp * HD], BF16)
            den_sb = singles.tile([P, ngrp * H], F32)
            ksv = ks_sb[:, 0:1]
            pb = parb[:, 0:1]

            # ------------------------------------------------ stage emitters
            qv_tiles = {}
            prod_tiles = {}
            ps_tiles = {}
            w_tiles = {}
            pa_tiles = {}

            def s0_dma(i):
                (g0, g1, K) = runs[i]
                R = g1 - g0
                RK = R * K
                r0 = int(colbase[g0]) * P
                qv_t = qvp.tile([P, RKMAX * 2 * HD], BF16, tag="qv")
                qv_tiles[i] = qv_t
                if R >= 2:
                    rh = R // 2
                    pieces = [(0, 64, 0, rh), (64, 64, 0, rh),
                              (0, 64, rh, R - rh), (64, 64, rh, R - rh)]
                else:
                    pieces = [(32 * q, 32, 0, 1) for q in range(4)]
                for pi, (p0, pn, ra, rn) in enumerate(pieces):
                    eng = nc.sync if pi % 2 == 0 else nc.scalar
                    in_ap = bass.AP(
                        tensor=tab_d.tensor,
                        offset=(tab_d.offset + r0 * 2 * HD
                                + p0 * K * 2 * HD + ra * P * K * 2 * HD),
                        ap=[[K * 2 * HD, pn], [P * K * 2 * HD, rn],
                            [1, K * 2 * HD]])
                    eng.dma_start(
                        out=qv_t[p0:p0 + pn,
                                 ra * K * 2 * HD:(ra + rn) * K * 2 * HD],
                        in_=in_ap)

            def s1_prod(i):
                (g0, g1, K) = runs[i]
                R = g1 - g0
                RK = R * K
                qv0 = qv_tiles[i][:, 0:1]
                prod = prodp.tile([P, RKMAX * HD], BF16, tag="prod")
                prod_tiles[i] = prod
                pv = prod[:, :RK * HD]
                p2 = bass.AP(tensor=pv.tensor, offset=pv.offset,
                             ap=[pv.ap[0], [HD, RK], [1, HD]])
                q2 = bass.AP(tensor=qv0.tensor, offset=qv0.offset,
                             ap=[qv0.ap[0], [2 * HD, RK], [1, HD]])
                kb = bass.AP(tensor=ksv.tensor, offset=ksv.offset + g0 * 2 * HD,
                             ap=[ksv.ap[0], [2 * HD, R], [0, K], [1, HD]])
                nc.vector.tensor_tensor(out=p2, in0=q2, in1=kb, op=ALU.mult)

            def s2_score(i):
                (g0, g1, K) = runs[i]
                RK = (g1 - g0) * K
                pv = prod_tiles[i][:, :RK * HD]
                ps = psS.tile([P, RKMAX * H], F32, tag="ps")
                ps_tiles[i] = ps
                sv = ps[:, :RK * H]
                for d in range(D):
                    rhs = bass.AP(tensor=pv.tensor, offset=pv.offset + d * H,
                                  ap=[pv.ap[0], [HD, RK], [1, H]])
                    nc.tensor.matmul(out=sv, lhsT=ident[:], rhs=rhs,
                                     start=(d == 0), stop=(d == D - 1))

            def s3_exp(i):
                (g0, g1, K) = runs[i]
                RK = (g1 - g0) * K
                sv = ps_tiles[i][:, :RK * H]
                w_t = wp.tile([P, RKMAX * 68], BF16, tag="w")
                w_tiles[i] = w_t
                wv = w_t[:, 0:1]
                exo = bass.AP(tensor=wv.tensor, offset=wv.offset + HD,
                              ap=[wv.ap[0], [68, RK], [1, H]])
                nc.scalar.activation(out=exo, in_=sv, func=ACTF.Exp,
                                     scale=0.25)

            def s4_w(i):
                (g0, g1, K) = runs[i]
                RK = (g1 - g0) * K
                qv0 = qv_tiles[i][:, 0:1]
                wv = w_tiles[i][:, 0:1]
                wo = bass.AP(tensor=wv.tensor, offset=wv.offset,
                             ap=[wv.ap[0], [68, RK], [1, HD]])
                vo = bass.AP(tensor=qv0.tensor, offset=qv0.offset + HD,
                             ap=[qv0.ap[0], [2 * HD, RK], [1, HD]])
                eb = bass.AP(tensor=wv.tensor, offset=wv.offset + HD,
                             ap=[wv.ap[0], [68, RK], [0, D], [1, H]])
                nc.vector.tensor_tensor(out=wo, in0=vo, in1=eb, op=ALU.mult)

            def s5_agg(i):
                (g0, g1, K) = runs[i]
                R = g1 - g0
                wv = w_tiles[i][:, 0:1]
                # pre-fold k-pairs on DVE for runs whose matmuls are small
                h = 0 if R >= 3 else 2
                cnt, step = K, 1
                for _ in range(h):
                    pairs = cnt // 2
                    if pairs == 0:
                        break
                    s68 = step * 68
                    lo = bass.AP(tensor=wv.tensor, offset=wv.offset,
                                 ap=[wv.ap[0], [68 * K, R], [2 * s68, pairs],
                                     [1, 68]])
                    hi = bass.AP(tensor=wv.tensor, offset=wv.offset + s68,
                                 ap=[wv.ap[0], [68 * K, R], [2 * s68, pairs],
                                     [1, 68]])
                    nc.vector.tensor_tensor(out=lo, in0=lo, in1=hi,
                                            op=ALU.add)
                    cnt = (cnt + 1) // 2
                    step *= 2
                pa_t = psA.tile([P, RMAX * 68], F32, tag="pa")
                pa_tiles[i] = pa_t
                av = pa_t[:, :R * 68]
                for k in range(cnt):
                    rhs = bass.AP(tensor=wv.tensor,
                                  offset=wv.offset + k * step * 68,
                                  ap=[wv.ap[0], [68 * K, R], [1, 68]])
                    nc.tensor.matmul(out=av, lhsT=ident[:], rhs=rhs,
                                     start=(k == 0), stop=(k == cnt - 1))

            def s6_copy(i):
                (g0, g1, K) = runs[i]
                R = g1 - g0
                av = pa_tiles[i][:, :R * 68]
                nc.scalar.activation(
                    out=bass.AP(tensor=agg_sb[:].tensor,
                                offset=agg_sb[:].offset + g0 * HD,
                                ap=[agg_sb[:].ap[0], [HD, R], [1, HD]]),
                    in_=bass.AP(tensor=av.tensor, offset=av.offset,
                                ap=[av.ap[0], [68, R], [1, HD]]),
                    func=ACTF.Copy)
                nc.scalar.activation(
                    out=bass.AP(tensor=den_sb[:].tensor,
                                offset=den_sb[:].offset + g0 * H,
                                ap=[den_sb[:].ap[0], [H, R], [1, H]]),
                    in_=bass.AP(tensor=av.tensor, offset=av.offset + HD,
                                ap=[av.ap[0], [68, R], [1, H]]),
                    func=ACTF.Copy)

            # -------------------------------------------- node-phase chunks
            # each chunk is emitted in 3 parts across consecutive iterations
            # so its cross-engine hops overlap a full edge run each.
            cstate = {}

            def node_a(ci):
                (_, _, ga, gb) = chunks[ci]
                G = gb - ga
                F = G * HD
                st = {}
                cstate[ci] = st
                dv = den_sb[:, ga * H:gb * H]
                nc.vector.tensor_tensor(out=dv, in0=dv,
                                        in1=corr_sb[:, ga * H:gb * H],
                                        op=ALU.add)
                nc.vector.reciprocal_approx_fast(out=dv, in_=dv)
                dinv = nodep.tile([P, RMAX * 2 * H], BF16, tag="dinv")
                nc.vector.tensor_scalar(out=dinv[:, :G * H], in0=dv,
                                        scalar1=1.0, scalar2=None,
                                        op0=ALU.mult)
                rst = nodep.tile([P, RMAX * 2 * HD], BF16, tag="rst")
                rv = rst[:, :F]
                r3 = bass.AP(tensor=rv.tensor, offset=rv.offset,
                             ap=[rv.ap[0], [HD, G], [1, HD]])
                a0 = agg_sb[:, 0:1]
                a3 = bass.AP(tensor=a0.tensor, offset=a0.offset + ga * HD,
                             ap=[a0.ap[0], [HD, G], [1, HD]])
                dq = dinv[:, 0:1]
                dinb = bass.AP(tensor=dq.tensor, offset=dq.offset,
                               ap=[dq.ap[0], [H, G], [0, D], [1, H]])
                nc.vector.tensor_tensor(out=r3, in0=a3, in1=dinb, op=ALU.mult)

                skipb = bass.AP(tensor=ksv.tensor,
                                offset=ksv.offset + ga * 2 * HD + HD,
                                ap=[ksv.ap[0], [2 * HD, G], [1, HD]])
                wg2b = bass.AP(tensor=pb.tensor, offset=pb.offset,
                               ap=[pb.ap[0], [0, G], [1, HD]])
                z = nodep.tile([P, RMAX * 2 * HD], BF16, tag="z")
                zv = z[:, :F]
                z3 = bass.AP(tensor=zv.tensor, offset=zv.offset,
                             ap=[zv.ap[0], [HD, G], [1, HD]])
                sc = nodep.tile([P, 8 * RMAX * 2], F32, tag="sc")
                r2 = sc[:, 0:G]
                gl = sc[:, G:2 * G]
                # gate logit: r2 = sum(rst*wg2'), gate = .5 + .5*tanh(...)
                nc.vector.tensor_tensor(out=z3, in0=r3, in1=wg2b, op=ALU.mult)
                nc.vector.tensor_reduce(out=r2, in_=z3, axis=AX.X, op=ALU.add)
                nc.vector.tensor_tensor(out=gl, in0=r2, in1=r1_sb[:, ga:gb],
                                        op=ALU.add)
                nc.scalar.activation(out=gl, in_=gl, func=ACTF.Tanh,
                                     scale=0.5, bias=bg2)
                gate = nodep.tile([P, RMAX * 2], BF16, tag="gate")
                nc.vector.tensor_scalar(out=gate[:, :G], in0=gl, scalar1=0.5,
                                        scalar2=0.5, op0=ALU.mult, op1=ALU.add)
                # dif = skip - rst (gpsimd), dif *= gate
                dif = nodep.tile([P, RMAX * 2 * HD], BF16, tag="dif")
                dv3 = bass.AP(tensor=dif[:].tensor, offset=dif[:].offset,
                              ap=[dif[:].ap[0], [HD, G], [1, HD]])
                nc.vector.tensor_tensor(out=dv3, in0=skipb, in1=r3,
                                        op=ALU.subtract)
                gq = gate[:, 0:1]
                gb_ = bass.AP(tensor=gq.tensor, offset=gq.offset,
                              ap=[gq.ap[0], [1, G], [0, HD]])
                nc.gpsimd.tensor_tensor(out=dv3, in0=dv3, in1=gb_,
                                        op=ALU.mult)
                st.update(G=G, F=F, rst=rst, rv=rv, r3=r3, z=z, zv=zv, z3=z3,
                          sc=sc, dif=dif)

            def node_b(ci):
                st = cstate[ci]
                G, F, rv, r3, zv, z3, sc = (st["G"], st["F"], st["rv"],
                                            st["r3"], st["zv"], st["z3"],
                                            st["sc"])
                vs = sc[:, 2 * G:3 * G]
                xh = sc[:, 3 * G:4 * G]
                t1 = sc[:, 4 * G:5 * G]
                mu = sc[:, 5 * G:6 * G]
                nc.vector.tensor_tensor(out=rv, in0=rv, in1=st["dif"][:, :F],
                                        op=ALU.add)
                # LayerNorm: mean, variance
                nc.vector.tensor_reduce(out=mu, in_=r3, axis=AX.X, op=ALU.add)
                mub = bass.AP(tensor=sc[:].tensor,
                              offset=sc[:].offset + 5 * G,
                              ap=[sc[:].ap[0], [1, G], [0, HD]])
                nc.vector.scalar_tensor_tensor(out=rv, in0=mub,
                                               scalar=-1.0 / HD, in1=rv,
                                               op0=ALU.mult, op1=ALU.add)
                nc.scalar.activation(out=zv, in_=rv, func=ACTF.Square)
                nc.vector.tensor_reduce(out=vs, in_=z3, axis=AX.X, op=ALU.add)
                nc.vector.tensor_scalar(out=vs, in0=vs, scalar1=1.0 / HD,
                                        scalar2=1e-5, op0=ALU.mult,
                                        op1=ALU.add)
                # quake rsqrt: y0 = bits(MAGIC - (i >> 1)); 2 Newton steps
                nc.vector.tensor_scalar(out=xh, in0=vs, scalar1=0.5,
                                        scalar2=None, op0=ALU.mult)
                vi = vs.bitcast(I32)
                nc.vector.tensor_scalar(out=vi, in0=vi, scalar1=1,
                                        scalar2=None,
                                        op0=ALU.logical_shift_right)
                nc.vector.tensor_scalar(out=vi, in0=vi, scalar1=-1,
                                        scalar2=MAGIC, op0=ALU.mult,
                                        op1=ALU.add)
                for _ in range(2):
                    nc.vector.tensor_tensor(out=t1, in0=vs, in1=vs,
                                            op=ALU.mult)
                    nc.vector.tensor_tensor(out=t1, in0=t1, in1=xh,
                                            op=ALU.mult)
                    nc.vector.tensor_scalar(out=t1, in0=t1, scalar1=-1.0,
                                            scalar2=1.5, op0=ALU.mult,
                                            op1=ALU.add)
                    nc.vector.tensor_tensor(out=vs, in0=vs, in1=t1,
                                            op=ALU.mult)
                isd = nodep.tile([P, RMAX * 2], BF16, tag="isd")
                nc.vector.tensor_scalar(out=isd[:, :G], in0=vs, scalar1=1.0,
                                        scalar2=None, op0=ALU.mult)
                st["isd"] = isd

            def node_c(ci):
                (_, _, ga, gb) = chunks[ci]
                st = cstate[ci]
                G, rv, r3 = st["G"], st["rv"], st["r3"]
                iq = st["isd"][:, 0:1]
                isb = bass.AP(tensor=iq.tensor, offset=iq.offset,
                              ap=[iq.ap[0], [1, G], [0, HD]])
                nc.gpsimd.tensor_tensor(out=rv, in0=rv, in1=isb, op=ALU.mult)
                if not trivial_affine:
                    gammab = bass.AP(tensor=pb.tensor, offset=pb.offset + HD,
                                     ap=[pb.ap[0], [0, G], [1, HD]])
                    betab = bass.AP(tensor=pb.tensor,
                                    offset=pb.offset + 2 * HD,
                                    ap=[pb.ap[0], [0, G], [1, HD]])
                    nc.gpsimd.tensor_tensor(out=r3, in0=r3, in1=gammab,
                                            op=ALU.mult)
                    nc.vector.tensor_tensor(out=r3, in0=r3, in1=betab,
                                            op=ALU.add)
                nc.scalar.activation(out=rv, in_=rv, func=ACTF.Prelu,
                                     alpha=pa)
                nc.gpsimd.dma_start(out=out_d[:, ga * HD:gb * HD], in_=rv)

            # ------------------------------------------------ emission loop
            part_a = {b: ci for ci, (a, b, _, _) in enumerate(chunks)}
            part_b = {b + 1: ci for ci, (a, b, _, _) in enumerate(chunks)}
            part_c = {b + 2: ci for ci, (a, b, _, _) in enumerate(chunks)}
            for i in range(nruns + 3):
                if i < nruns:
                    s0_dma(i)
                    s1_prod(i)
                    s2_score(i)
                    s3_exp(i)
                if 0 < i <= nruns:
                    s4_w(i - 1)
                    s5_agg(i - 1)
                    s6_copy(i - 1)
                if i in part_a:
                    node_a(part_a[i])
                if i in part_b:
                    node_b(part_b[i])
                if i in part_c:
                    node_c(part_c[i])

    nc.compile()
    return nc


# ------------------------------------------------------------------- driver

_CACHE = {}


def _get_nc(plan, ncores, trivial_affine):
    key = (tuple(plan["Kpad"].tolist()), plan["grid"], ncores, trivial_affine)
    if key not in _CACHE:
        _CACHE[key] = _build_nc(plan, ncores, trivial_affine)
    return _CACHE[key]


def _make_inmaps(plan, params, ncores):
    (Wk, bk, Wskip, bskip, Wgate, bgate, ln_gamma, ln_beta, prelu_a) = params
    cm = _dh_perm()
    wg = np.asarray(Wgate, np.float32).reshape(3 * HD)
    wg1 = wg[0:64] + wg[128:192]          # acts on skip
    wg2 = wg[64:128] - wg[128:192]        # acts on rst

    wk = np.concatenate([np.asarray(Wk, np.float32),
                         np.asarray(bk, np.float32).reshape(1, HD)])[:, cm]
    wsk_f = np.concatenate([np.asarray(Wskip, np.float32),
                            np.asarray(bskip, np.float32).reshape(1, HD)])
    wsk = wsk_f[:, cm]
    wkws = np.zeros((IN_F + 1, 132), np.float32)
    wkws[:, 0:HD] = wk
    wkws[:, HD:2 * HD] = wsk
    wkws[:, 128] = wsk_f @ wg1            # r1 column (skip-side gate logit)
    wkws = wkws.astype(BF)

    parb = np.zeros((1, 3 * HD), np.float32)
    parb[0, 0:HD] = wg2[cm]
    parb[0, HD:2 * HD] = np.asarray(ln_gamma, np.float32)[cm]
    parb[0, 2 * HD:3 * HD] = np.asarray(ln_beta, np.float32)[cm]
    parb = parb.astype(BF)
    parf = np.array([[np.float32(np.asarray(bgate).reshape(-1)[0]) * 0.5,
                      np.float32(np.asarray(prelu_a).reshape(-1)[0]),
                      0.0]], np.float32)

    in_maps = []
    for c in range(ncores):
        pc = plan["per_core"][c]
        m = dict(featT=plan["featTs"][c], tab=pc["tab"], corr=pc["corr"],
                 ident=plan["ident"], wkws=wkws, parb=parb, parf=parf)
        in_maps.append(m)
    return in_maps


def run(q_src, v_src, feat, src, dst, Wk, bk, Wskip, bskip, Wgate, bgate,
        ln_gamma, ln_beta, prelu_a, ncores=NCORES, trace=False):
    plan = _plan(q_src, v_src, feat, src, dst, ncores)
    trivial_affine = bool(
        np.all(np.asarray(ln_gamma, np.float32) == 1.0)
        and np.all(np.asarray(ln_beta, np.float32) == 0.0))
    nc = _get_nc(plan, ncores, trivial_affine)
    in_maps = _make_inmaps(
        plan, (Wk, bk, Wskip, bskip, Wgate, bgate, ln_gamma, ln_beta, prelu_a),
        ncores)
    res = run_bass_kernel_spmd(nc, in_maps, core_ids=list(range(ncores)),
                               trace=trace)
    n, npc, ngrp = plan["n"], plan["npc"], plan["ngrp"]
    out = np.empty((n, HD), np.float32)
    for c in range(ncores):
        r = np.asarray(res.results[c]["out"]).astype(np.float32)
        # [P, ngrp, D, H] -> [ngrp, P, H, D] -> [grid, HD]
        arr = r.reshape(P, ngrp, D, H).transpose(1, 0, 3, 2).reshape(-1, HD)
        out[c * npc + plan["cores"][c]["perm"]] = \
            arr[plan["ndum"]:plan["ndum"] + npc]
    return out, res, plan, in_maps, nc


def kernel(**inputs):
    out, _, _, _, _ = run(**inputs)
    return out


# revision 47
# speedup vs baseline: 1.0408x; 1.0177x over previous
"""Trainium2 Bass kernel for nn_DenTargetTransformerConv (GNN message passing).

Strategy (graph/data parallel, dst-owner sharding across 8 NeuronCores):
  - Nodes are partitioned by dst-id range; each core owns N/8 nodes and all
    edges whose dst falls in its range.  The halo exchange of src features is
    materialized host-side as a per-core edge-expanded bf16 table (one
    256-byte [q|v] row per edge slot, rows laid out in consumption order), so
    the device reads it with plain strided hardware-DGE DMAs at full
    bandwidth - no gathers.  Each run's table slice is fetched as four
    pieces alternating between the SP and Activation HWDGE queue pools so
    ~12 DMA queues stay busy across the three in-flight runs.  Runs are
    processed big-K first so their poorly-amortized work hides under the
    DMA-bound phase and the cheap small-K runs form the tail.
  - Per core, own nodes are sorted by in-degree and packed into groups of
    128 (SBUF partition dim).  Consecutive groups are merged into runs of
    R <= 7 groups sharing a padded slot count Kp (R*Kp <= 96); all 8 cores
    share one compiled program.
  - All edge-phase math is bf16 with (d,h)-minor layouts so every
    tensor_tensor hits the DVE 2x mode.  The two big reductions (score
    over d, aggregation over k) run on the tensor engine as identity-matmul
    slice accumulations into PSUM (errata-free 2.4 GHz, overlapped with
    DVE); the softmax denominator rides along in the same accumulation as 4
    extra columns, and the skip-side gate logit rides along in the k/skip
    linear as a 129th column.  Edge-softmax max-subtraction is elided
    (scores are O(+-5)).  Padded slots contribute exp(0)=1 to the
    denominator; a host-computed per-node correction (deg - Kp) fixes it.
  - Emission is software-pipelined one stage deep (run i's DMA/prod/score
    before run i-1's w/agg/copyout) so no engine head-of-line blocks on a
    cross-engine dependency.  The node phase (gate + LayerNorm + PReLU) is
    processed in chunks of ~2 runs woven between edge runs, so only the
    last chunk's short serial chain sits after the final run.  Everything
    scalar-engine-side (exp, tanh-sigmoid, square, prelu, copies) lives in
    the one `exp_and_others` activation table - zero table reloads; rsqrt
    for LayerNorm is a quake-style bit hack + 2 Newton steps on tiny
    [128, G] tiles on DVE.
"""

import numpy as np
import ml_dtypes

import concourse.bacc as bacc
import concourse.bass as bass
import concourse.tile as tile
from concourse import mybir
from concourse.bass_utils import run_bass_kernel_spmd

F32 = mybir.dt.float32
I32 = mybir.dt.int32
BF16 = mybir.dt.bfloat16
FP8 = mybir.dt.float8e4
BF = ml_dtypes.bfloat16
F8 = ml_dtypes.float8_e4m3
AX = mybir.AxisListType
ALU = mybir.AluOpType
ACTF = mybir.ActivationFunctionType

P = 128
NCORES = 8
HD = 64          # H * D
H, D = 4, 16
IN_F = 64

RMAX = 7         # groups per run  (R*68 <= 476 PSUM f32 bank)
RKMAX = 96       # slot-columns per run (SBUF tiles)
MAGIC = 0x5F3759DF


# ----------------------------------------------------------------- host prep

def _plan(q_src, v_src, feat, src, dst, ncores):
    n = feat.shape[0]
    npc = n // ncores
    ngrp = (npc + P - 1) // P
    grid = ngrp * P
    ndum = grid - npc

    # per-node [q | v] rows in (d, h)-minor order, bf16
    q2 = np.asarray(q_src, np.float32).reshape(n, H, D).transpose(0, 2, 1)
    v2 = np.asarray(v_src, np.float32).reshape(n, H, D).transpose(0, 2, 1)
    qv = np.concatenate([q2.reshape(n, HD), v2.reshape(n, HD)],
                        axis=1).astype(BF)          # [n, 128]

    src = np.asarray(src).astype(np.int64)
    dst = np.asarray(dst).astype(np.int64)
    order = np.argsort(dst, kind="stable")
    dst_s, src_s = dst[order], src[order]
    bounds = np.searchsorted(dst_s, np.arange(ncores + 1) * npc)

    cores = []
    gmax = np.zeros((ncores, ngrp), np.int64)
    gdegs = []
    for c in range(ncores):
        lo, hi = bounds[c], bounds[c + 1]
        dstL = dst_s[lo:hi] - c * npc          # ascending
        srcL = src_s[lo:hi]
        deg = np.bincount(dstL, minlength=npc)
        starts = np.concatenate([[0], np.cumsum(deg)])
        rank = np.arange(len(dstL)) - starts[dstL]
        perm = np.argsort(deg, kind="stable")  # ascending degree
        pos_of = np.empty(npc, np.int64)
        pos_of[perm] = ndum + np.arange(npc)
        gd = np.zeros(grid, np.int64)
        gd[ndum:] = deg[perm]
        gdeg = gd.reshape(ngrp, P)             # [g, p]
        gmax[c] = gdeg.max(1)
        gdegs.append(gdeg)
        cores.append(dict(dstL=dstL, srcL=srcL, rank=rank, perm=perm,
                          pos_of=pos_of))

    K = np.maximum(gmax.max(0), 1)             # shared per-group slot count

    # runs of consecutive groups, padded to the run max Kp
    runs = []       # (g0, g1, Kp)
    g = 0
    while g < ngrp:
        ge = g + 1
        while (ge < ngrp and ge - g < RMAX
               and (ge - g + 1) * K[ge] <= RKMAX
               and K[ge] <= K[g] * 1.3 + 2):
            ge += 1
        runs.append((g, ge, int(K[ge - 1])))
        g = ge
    Kpad = np.zeros(ngrp, np.int64)
    for (g0, g1, kp) in runs:
        Kpad[g0:g1] = kp
    assert Kpad.max() <= RKMAX

    colbase = np.concatenate([[0], np.cumsum(Kpad)]).astype(np.int64)
    totc = int(colbase[-1])

    # process runs big-K first: their small-R work hides under the
    # DMA-bound phase and the cheap small-K runs land in the tail
    porder = list(reversed(range(len(runs))))

    # per-core tab + denominator correction
    per_core = []
    for c in range(ncores):
        cd = cores[c]
        pos_e = cd["pos_of"][cd["dstL"]]       # grid position of each edge
        g_e = pos_e // P
        p_e = pos_e % P
        tab = np.zeros((totc * P, 2 * HD), BF)
        rows = colbase[g_e] * P + p_e * Kpad[g_e] + cd["rank"]
        tab[rows] = qv[cd["srcL"]]
        corr = (gdegs[c].T.astype(np.float32) -
                Kpad[None, :].astype(np.float32)) + 1e-9       # [P, ngrp]
        corr_w = np.repeat(corr, H, axis=1).astype(np.float32)  # [P, ngrp*H]
        per_core.append(dict(tab=tab, corr=corr_w))

    # featT with ones row, per core, grid-permuted: [IN_F+1, grid] bf16
    featTs = []
    feat = np.asarray(feat, np.float32)
    for c in range(ncores):
        ft = np.zeros((IN_F + 1, grid), np.float32)
        ft[IN_F, :] = 1.0
        perm = cores[c]["perm"]
        ft[:IN_F, ndum:] = feat[c * npc + perm].T
        featTs.append(ft.astype(BF))

    ident = np.eye(P, dtype=F8)

    return dict(n=n, npc=npc, ngrp=ngrp, grid=grid, ndum=ndum, K=K,
                Kpad=Kpad, colbase=colbase, totc=totc, runs=runs,
                porder=porder, ident=ident, cores=cores, per_core=per_core,
                featTs=featTs)


# standard (h,d) column index -> (d,h) position
def _dh_perm():
    cm = np.empty(HD, np.int64)
    for d in range(D):
        for h in range(H):
            cm[d * H + h] = h * D + d
    return cm            # newcol j takes oldcol cm[j]


# ------------------------------------------------------------- device build

def _build_nc(plan, ncores, trivial_affine=False):
    ngrp = plan["ngrp"]
    grid = plan["grid"]
    colbase = plan["colbase"]
    NG = ngrp
    runs = [plan["runs"][j] for j in plan["porder"]]   # processing order
    nruns = len(runs)
    # node-phase chunks over the processing order: pairs early, singles at
    # the end so the final serial chains are tiny
    chunks = []     # (run_lo, run_hi, g_lo, g_hi)
    i = 0
    while i < nruns:
        j = min(i + 2, nruns) if i < nruns - 3 else i + 1
        chunks.append((i, j, min(r[0] for r in runs[i:j]),
                       max(r[1] for r in runs[i:j])))
        i = j

    nc = bacc.Bacc("TRN2", target_bir_lowering=False, debug=False,
                   num_devices=ncores)

    featT_d = nc.dram_tensor("featT", [IN_F + 1, grid], BF16,
                             kind="ExternalInput").ap()
    wkws_d = nc.dram_tensor("wkws", [IN_F + 1, 132], BF16,
                            kind="ExternalInput").ap()
    tab_d = nc.dram_tensor("tab", [plan["totc"] * P, 2 * HD], BF16,
                           kind="ExternalInput").ap()
    corr_d = nc.dram_tensor("corr", [P, ngrp * H], F32,
                            kind="ExternalInput").ap()
    ident_d = nc.dram_tensor("ident", [P, P], FP8, kind="ExternalInput").ap()
    # bf16 params row: [wg2'(64) | gamma(64) | beta(64)]
    parb_d = nc.dram_tensor("parb", [1, 3 * HD], BF16,
                            kind="ExternalInput").ap()
    # f32 params row: [bgate/2, prelu_a, unused]
    parf_d = nc.dram_tensor("parf", [1, 3], F32, kind="ExternalInput").ap()
    out_d = nc.dram_tensor("out", [P, ngrp * HD], BF16,
                           kind="ExternalOutput").ap()

    with tile.TileContext(nc) as tc:
        with (
            tc.tile_pool(name="singles", bufs=1) as singles,
            tc.tile_pool(name="psL", bufs=2, space="PSUM") as psL,
            tc.tile_pool(name="psS", bufs=2, space="PSUM") as psS,
            tc.tile_pool(name="psA", bufs=3, space="PSUM") as psA,
            tc.tile_pool(name="qvp", bufs=3) as qvp,
            tc.tile_pool(name="prodp", bufs=2) as prodp,
            tc.tile_pool(name="wp", bufs=3) as wp,
            tc.tile_pool(name="nodep", bufs=2) as nodep,
        ):
            # ---- static loads (featT split 4-ways across both HWDGE pools)
            featT = singles.tile([IN_F + 1, grid], BF16)
            fq = grid // 4
            for fi in range(4):
                eng = nc.sync if fi % 2 == 0 else nc.scalar
                eng.dma_start(out=featT[:, fi * fq:(fi + 1) * fq],
                              in_=featT_d[:, fi * fq:(fi + 1) * fq])
            wkws = singles.tile([IN_F + 1, 132], BF16)
            nc.sync.dma_start(out=wkws[:], in_=wkws_d[:])
            ident = singles.tile([P, P], FP8)
            nc.scalar.dma_start(out=ident[:], in_=ident_d[:])
            corr_sb = singles.tile([P, ngrp * H], F32)
            nc.sync.dma_start(out=corr_sb[:], in_=corr_d[:])
            parb = singles.tile([P, 3 * HD], BF16)
            nc.gpsimd.dma_start(
                out=parb[:],
                in_=bass.AP(tensor=parb_d.tensor, offset=parb_d.offset,
                            ap=[[0, P], [1, 3 * HD]]))
            parf = singles.tile([P, 3], F32)
            nc.gpsimd.dma_start(
                out=parf[:],
                in_=bass.AP(tensor=parf_d.tensor, offset=parf_d.offset,
                            ap=[[0, P], [1, 3]]))
            bg2 = parf[:, 0:1]
            pa = parf[:, 1:2]

            # ---- per-node linears on PE: ks[:, g*128:...] = [k | skip],
            # plus the skip-side gate logit r1 as column 128.
            ks_sb = singles.tile([P, ngrp * 2 * HD], BF16)
            r1_sb = singles.tile([P, ngrp], F32)
            for q0 in reversed(range(0, ngrp, 3)):
                q1 = min(q0 + 3, ngrp)
                nq = q1 - q0
                pl = psL.tile([P, 3 * 132], F32, tag="pl")
                for g in range(q0, q1):
                    nc.tensor.matmul(out=pl[:, (g - q0) * 132:(g - q0 + 1) * 132],
                                     lhsT=featT[:, g * P:(g + 1) * P],
                                     rhs=wkws[:], start=True, stop=True)
                plv = pl[:, 0:1]
                nc.scalar.activation(
                    out=bass.AP(tensor=ks_sb[:].tensor,
                                offset=ks_sb[:].offset + q0 * 2 * HD,
                                ap=[ks_sb[:].ap[0], [2 * HD, nq], [1, 2 * HD]]),
                    in_=bass.AP(tensor=plv.tensor, offset=plv.offset,
                                ap=[plv.ap[0], [132, nq], [1, 2 * HD]]),
                    func=ACTF.Copy)
                nc.scalar.activation(
                    out=r1_sb[:, q0:q1],
                    in_=bass.AP(tensor=plv.tensor, offset=plv.offset + 128,
                                ap=[plv.ap[0], [132, nq], [1, 1]]),
                    func=ACTF.Copy)

            agg_sb = singles.tile([P, ngrp * HD], BF16)
            den_sb = singles.tile([P, ngrp * H], F32)
            ksv = ks_sb[:, 0:1]
            pb = parb[:, 0:1]

            # ------------------------------------------------ stage emitters
            qv_tiles = {}
            prod_tiles = {}
            ps_tiles = {}
            w_tiles = {}
            pa_tiles = {}

            def s0_dma(i):
                (g0, g1, K) = runs[i]
                R = g1 - g0
                RK = R * K
                r0 = int(colbase[g0]) * P
                qv_t = qvp.tile([P, RKMAX * 2 * HD], BF16, tag="qv")
                qv_tiles[i] = qv_t
                if R >= 2:
                    rh = R // 2
                    pieces = [(0, 64, 0, rh), (64, 64, 0, rh),
                              (0, 64, rh, R - rh), (64, 64, rh, R - rh)]
                else:
                    pieces = [(32 * q, 32, 0, 1) for q in range(4)]
                for pi, (p0, pn, ra, rn) in enumerate(pieces):
                    eng = nc.sync if pi % 2 == 0 else nc.scalar
                    in_ap = bass.AP(
                        tensor=tab_d.tensor,
                        offset=(tab_d.offset + r0 * 2 * HD
                                + p0 * K * 2 * HD + ra * P * K * 2 * HD),
                        ap=[[K * 2 * HD, pn], [P * K * 2 * HD, rn],
                            [1, K * 2 * HD]])
                    eng.dma_start(
                        out=qv_t[p0:p0 + pn,
                                 ra * K * 2 * HD:(ra + rn) * K * 2 * HD],
                        in_=in_ap)

            def s1_prod(i):
                (g0, g1, K) = runs[i]
                R = g1 - g0
                RK = R * K
                qv0 = qv_tiles[i][:, 0:1]
                prod = prodp.tile([P, RKMAX * HD], BF16, tag="prod")
                prod_tiles[i] = prod
                pv = prod[:, :RK * HD]
                p2 = bass.AP(tensor=pv.tensor, offset=pv.offset,
                             ap=[pv.ap[0], [HD, RK], [1, HD]])
                q2 = bass.AP(tensor=qv0.tensor, offset=qv0.offset,
                             ap=[qv0.ap[0], [2 * HD, RK], [1, HD]])
                kb = bass.AP(tensor=ksv.tensor, offset=ksv.offset + g0 * 2 * HD,
                             ap=[ksv.ap[0], [2 * HD, R], [0, K], [1, HD]])
                nc.vector.tensor_tensor(out=p2, in0=q2, in1=kb, op=ALU.mult)

            def s2_score(i):
                (g0, g1, K) = runs[i]
                RK = (g1 - g0) * K
                pv = prod_tiles[i][:, :RK * HD]
                ps = psS.tile([P, RKMAX * H], F32, tag="ps")
                ps_tiles[i] = ps
                sv = ps[:, :RK * H]
                for d in range(D):
                    rhs = bass.AP(tensor=pv.tensor, offset=pv.offset + d * H,
                                  ap=[pv.ap[0], [HD, RK], [1, H]])
                    nc.tensor.matmul(out=sv, lhsT=ident[:], rhs=rhs,
                                     start=(d == 0), stop=(d == D - 1))

            def s3_exp(i):
                (g0, g1, K) = runs[i]
                RK = (g1 - g0) * K
                sv = ps_tiles[i][:, :RK * H]
                w_t = wp.tile([P, RKMAX * 68], BF16, tag="w")
                w_tiles[i] = w_t
                wv = w_t[:, 0:1]
                exo = bass.AP(tensor=wv.tensor, offset=wv.offset + HD,
                              ap=[wv.ap[0], [68, RK], [1, H]])
                nc.scalar.activation(out=exo, in_=sv, func=ACTF.Exp,
                                     scale=0.25)

            def s4_w(i):
                (g0, g1, K) = runs[i]
                RK = (g1 - g0) * K
                qv0 = qv_tiles[i][:, 0:1]
                wv = w_tiles[i][:, 0:1]
                wo = bass.AP(tensor=wv.tensor, offset=wv.offset,
                             ap=[wv.ap[0], [68, RK], [1, HD]])
                vo = bass.AP(tensor=qv0.tensor, offset=qv0.offset + HD,
                             ap=[qv0.ap[0], [2 * HD, RK], [1, HD]])
                eb = bass.AP(tensor=wv.tensor, offset=wv.offset + HD,
                             ap=[wv.ap[0], [68, RK], [0, D], [1, H]])
                nc.vector.tensor_tensor(out=wo, in0=vo, in1=eb, op=ALU.mult)

            def s5_agg(i):
                (g0, g1, K) = runs[i]
                R = g1 - g0
                wv = w_tiles[i][:, 0:1]
                # pre-fold k-pairs on DVE for runs whose matmuls are small
                h = 0 if R >= 3 else 2
                cnt, step = K, 1
                for _ in range(h):
                    pairs = cnt // 2
                    if pairs == 0:
                        break
                    s68 = step * 68
                    lo = bass.AP(tensor=wv.tensor, offset=wv.offset,
                                 ap=[wv.ap[0], [68 * K, R], [2 * s68, pairs],
                                     [1, 68]])
                    hi = bass.AP(tensor=wv.tensor, offset=wv.offset + s68,
                                 ap=[wv.ap[0], [68 * K, R], [2 * s68, pairs],
                                     [1, 68]])
                    nc.vector.tensor_tensor(out=lo, in0=lo, in1=hi,
                                            op=ALU.add)
                    cnt = (cnt + 1) // 2
                    step *= 2
                pa_t = psA.tile([P, RMAX * 68], F32, tag="pa")
                pa_tiles[i] = pa_t
                av = pa_t[:, :R * 68]
                for k in range(cnt):
                    rhs = bass.AP(tensor=wv.tensor,
                                  offset=wv.offset + k * step * 68,
                                  ap=[wv.ap[0], [68 * K, R], [1, 68]])
                    nc.tensor.matmul(out=av, lhsT=ident[:], rhs=rhs,
                                     start=(k == 0), stop=(k == cnt - 1))

            def s6_copy(i):
                (g0, g1, K) = runs[i]
                R = g1 - g0
                av = pa_tiles[i][:, :R * 68]
                nc.scalar.activation(
                    out=bass.AP(tensor=agg_sb[:].tensor,
                                offset=agg_sb[:].offset + g0 * HD,
                                ap=[agg_sb[:].ap[0], [HD, R], [1, HD]]),
                    in_=bass.AP(tensor=av.tensor, offset=av.offset,
                                ap=[av.ap[0], [68, R], [1, HD]]),
                    func=ACTF.Copy)
                nc.scalar.activation(
                    out=bass.AP(tensor=den_sb[:].tensor,
                                offset=den_sb[:].offset + g0 * H,
                                ap=[den_sb[:].ap[0], [H, R], [1, H]]),
                    in_=bass.AP(tensor=av.tensor, offset=av.offset + HD,
                                ap=[av.ap[0], [68, R], [1, H]]),
                    func=ACTF.Copy)

            # -------------------------------------------- node-phase chunks
            # each chunk is emitted in 3 parts across consecutive iterations
            # so its cross-engine hops overlap a full edge run each.
            cstate = {}

            def node_a(ci):
                (_, _, ga, gb) = chunks[ci]
                G = gb - ga
                F = G * HD
                st = {}
                cstate[ci] = st
                dv = den_sb[:, ga * H:gb * H]
                nc.vector.tensor_tensor(out=dv, in0=dv,
                                        in1=corr_sb[:, ga * H:gb * H],
                                        op=ALU.add)
                nc.vector.reciprocal_approx_fast(out=dv, in_=dv)
                dinv = nodep.tile([P, RMAX * 2 * H], BF16, tag="dinv")
                nc.vector.tensor_scalar(out=dinv[:, :G * H], in0=dv,
                                        scalar1=1.0, scalar2=None,
                                        op0=ALU.mult)
                rst = nodep.tile([P, RMAX * 2 * HD], BF16, tag="rst")
                rv = rst[:, :F]
                r3 = bass.AP(tensor=rv.tensor, offset=rv.offset,
                             ap=[rv.ap[0], [HD, G], [1, HD]])
                a0 = agg_sb[:, 0:1]
                a3 = bass.AP(tensor=a0.tensor, offset=a0.offset + ga * HD,
                             ap=[a0.ap[0], [HD, G], [1, HD]])
                dq = dinv[:, 0:1]
                dinb = bass.AP(tensor=dq.tensor, offset=dq.offset,
                               ap=[dq.ap[0], [H, G], [0, D], [1, H]])
                nc.vector.tensor_tensor(out=r3, in0=a3, in1=dinb, op=ALU.mult)

                skipb = bass.AP(tensor=ksv.tensor,
                                offset=ksv.offset + ga * 2 * HD + HD,
                                ap=[ksv.ap[0], [2 * HD, G], [1, HD]])
                wg2b = bass.AP(tensor=pb.tensor, offset=pb.offset,
                               ap=[pb.ap[0], [0, G], [1, HD]])
                z = nodep.tile([P, RMAX * 2 * HD], BF16, tag="z")
                zv = z[:, :F]
                z3 = bass.AP(tensor=zv.tensor, offset=zv.offset,
                             ap=[zv.ap[0], [HD, G], [1, HD]])
                sc = nodep.tile([P, 8 * RMAX * 2], F32, tag="sc")
                r2 = sc[:, 0:G]
                gl = sc[:, G:2 * G]
                # gate logit: r2 = sum(rst*wg2'), gate = .5 + .5*tanh(...)
                nc.vector.tensor_tensor(out=z3, in0=r3, in1=wg2b, op=ALU.mult)
                nc.vector.tensor_reduce(out=r2, in_=z3, axis=AX.X, op=ALU.add)
                nc.vector.tensor_tensor(out=gl, in0=r2, in1=r1_sb[:, ga:gb],
                                        op=ALU.add)
                nc.scalar.activation(out=gl, in_=gl, func=ACTF.Tanh,
                                     scale=0.5, bias=bg2)
                gate = nodep.tile([P, RMAX * 2], BF16, tag="gate")
                nc.vector.tensor_scalar(out=gate[:, :G], in0=gl, scalar1=0.5,
                                        scalar2=0.5, op0=ALU.mult, op1=ALU.add)
                # dif = skip - rst (gpsimd), dif *= gate
                dif = nodep.tile([P, RMAX * 2 * HD], BF16, tag="dif")
                dv3 = bass.AP(tensor=dif[:].tensor, offset=dif[:].offset,
                              ap=[dif[:].ap[0], [HD, G], [1, HD]])
                nc.vector.tensor_tensor(out=dv3, in0=skipb, in1=r3,
                                        op=ALU.subtract)
                gq = gate[:, 0:1]
                gb_ = bass.AP(tensor=gq.tensor, offset=gq.offset,
                              ap=[gq.ap[0], [1, G], [0, HD]])
                nc.gpsimd.tensor_tensor(out=dv3, in0=dv3, in1=gb_,
                                        op=ALU.mult)
                st.update(G=G, F=F, rst=rst, rv=rv, r3=r3, z=z, zv=zv, z3=z3,
                          sc=sc, dif=dif)

            def node_b(ci):
                st = cstate[ci]
                G, F, rv, r3, zv, z3, sc = (st["G"], st["F"], st["rv"],
                                            st["r3"], st["zv"], st["z3"],
                                            st["sc"])
                vs = sc[:, 2 * G:3 * G]
                xh = sc[:, 3 * G:4 * G]
                t1 = sc[:, 4 * G:5 * G]
                mu = sc[:, 5 * G:6 * G]
                nc.vector.tensor_tensor(out=rv, in0=rv, in1=st["dif"][:, :F],
                                        op=ALU.add)
                # LayerNorm: mean, variance
                nc.vector.tensor_reduce(out=mu, in_=r3, axis=AX.X, op=ALU.add)
                mub = bass.AP(tensor=sc[:].tensor,
                              offset=sc[:].offset + 5 * G,
                              ap=[sc[:].ap[0], [1, G], [0, HD]])
                nc.vector.scalar_tensor_tensor(out=rv, in0=mub,
                                               scalar=-1.0 / HD, in1=rv,
                                               op0=ALU.mult, op1=ALU.add)
                nc.scalar.activation(out=zv, in_=rv, func=ACTF.Square)
                nc.vector.tensor_reduce(out=vs, in_=z3, axis=AX.X, op=ALU.add)
                nc.vector.tensor_scalar(out=vs, in0=vs, scalar1=1.0 / HD,
                                        scalar2=1e-5, op0=ALU.mult,
                                        op1=ALU.add)
                # quake rsqrt: y0 = bits(MAGIC - (i >> 1)); 2 Newton steps
                nc.vector.tensor_scalar(out=xh, in0=vs, scalar1=0.5,
                                        scalar2=None, op0=ALU.mult)
                vi = vs.bitcast(I32)
                nc.vector.tensor_scalar(out=vi, in0=vi, scalar1=1,
                                        scalar2=None,
                                        op0=ALU.logical_shift_right)
                nc.vector.tensor_scalar(out=vi, in0=vi, scalar1=-1,
                                        scalar2=MAGIC, op0=ALU.mult,
                                        op1=ALU.add)
                for _ in range(2):
                    nc.vector.tensor_tensor(out=t1, in0=vs, in1=vs,
                                            op=ALU.mult)
                    nc.vector.tensor_tensor(out=t1, in0=t1, in1=xh,
                                            op=ALU.mult)
                    nc.vector.tensor_scalar(out=t1, in0=t1, scalar1=-1.0,
                                            scalar2=1.5, op0=ALU.mult,
                                            op1=ALU.add)
                    nc.vector.tensor_tensor(out=vs, in0=vs, in1=t1,
                                            op=ALU.mult)
                isd = nodep.tile([P, RMAX * 2], BF16, tag="isd")
                nc.vector.tensor_scalar(out=isd[:, :G], in0=vs, scalar1=1.0,
                                        scalar2=None, op0=ALU.mult)
                st["isd"] = isd

            def node_c(ci):
                (_, _, ga, gb) = chunks[ci]
                st = cstate[ci]
                G, rv, r3 = st["G"], st["rv"], st["r3"]
                iq = st["isd"][:, 0:1]
                isb = bass.AP(tensor=iq.tensor, offset=iq.offset,
                              ap=[iq.ap[0], [1, G], [0, HD]])
                nc.gpsimd.tensor_tensor(out=rv, in0=rv, in1=isb, op=ALU.mult)
                if not trivial_affine:
                    gammab = bass.AP(tensor=pb.tensor, offset=pb.offset + HD,
                                     ap=[pb.ap[0], [0, G], [1, HD]])
                    betab = bass.AP(tensor=pb.tensor,
                                    offset=pb.offset + 2 * HD,
                                    ap=[pb.ap[0], [0, G], [1, HD]])
                    nc.gpsimd.tensor_tensor(out=r3, in0=r3, in1=gammab,
                                            op=ALU.mult)
                    nc.vector.tensor_tensor(out=r3, in0=r3, in1=betab,
                                            op=ALU.add)
                nc.scalar.activation(out=rv, in_=rv, func=ACTF.Prelu,
                                     alpha=pa)
                nc.gpsimd.dma_start(out=out_d[:, ga * HD:gb * HD], in_=rv)

            # ------------------------------------------------ emission loop
            part_a = {b: ci for ci, (a, b, _, _) in enumerate(chunks)}
            part_b = {b + 1: ci for ci, (a, b, _, _) in enumerate(chunks)}
            part_c = {b + 2: ci for ci, (a, b, _, _) in enumerate(chunks)}
            for i in range(nruns + 3):
                if i < nruns:
                    s0_dma(i)
                    s1_prod(i)
                    s2_score(i)
                    s3_exp(i)
                if 0 < i <= nruns:
                    s4_w(i - 1)
                    s5_agg(i - 1)
                    s6_copy(i - 1)
                if i in part_a:
                    node_a(part_a[i])
                if i in part_b:
                    node_b(part_b[i])
                if i in part_c:
                    node_c(part_c[i])

    nc.compile()
    return nc


# ------------------------------------------------------------------- driver

_CACHE = {}


def _get_nc(plan, ncores, trivial_affine):
    key = (tuple(plan["Kpad"].tolist()), plan["grid"], ncores, trivial_affine)
    if key not in _CACHE:
        _CACHE[key] = _build_nc(plan, ncores, trivial_affine)
    return _CACHE[key]


def _make_inmaps(plan, params, ncores):
    (Wk, bk, Wskip, bskip, Wgate, bgate, ln_gamma, ln_beta, prelu_a) = params
    cm = _dh_perm()
    wg = np.asarray(Wgate, np.float32).reshape(3 * HD)
    wg1 = wg[0:64] + wg[128:192]          # acts on skip
    wg2 = wg[64:128] - wg[128:192]        # acts on rst

    wk = np.concatenate([np.asarray(Wk, np.float32),
                         np.asarray(bk, np.float32).reshape(1, HD)])[:, cm]
    wsk_f = np.concatenate([np.asarray(Wskip, np.float32),
                            np.asarray(bskip, np.float32).reshape(1, HD)])
    wsk = wsk_f[:, cm]
    wkws = np.zeros((IN_F + 1, 132), np.float32)
    wkws[:, 0:HD] = wk
    wkws[:, HD:2 * HD] = wsk
    wkws[:, 128] = wsk_f @ wg1            # r1 column (skip-side gate logit)
    wkws = wkws.astype(BF)

    parb = np.zeros((1, 3 * HD), np.float32)
    parb[0, 0:HD] = wg2[cm]
    parb[0, HD:2 * HD] = np.asarray(ln_gamma, np.float32)[cm]
    parb[0, 2 * HD:3 * HD] = np.asarray(ln_beta, np.float32)[cm]
    parb = parb.astype(BF)
    parf = np.array([[np.float32(np.asarray(bgate).reshape(-1)[0]) * 0.5,
                      np.float32(np.asarray(prelu_a).reshape(-1)[0]),
                      0.0]], np.float32)

    in_maps = []
    for c in range(ncores):
        pc = plan["per_core"][c]
        m = dict(featT=plan["featTs"][c], tab=pc["tab"], corr=pc["corr"],
                 ident=plan["ident"], wkws=wkws, parb=parb, parf=parf)
        in_maps.append(m)
    return in_maps


def run(q_src, v_src, feat, src, dst, Wk, bk, Wskip, bskip, Wgate, bgate,
        ln_gamma, ln_beta, prelu_a, ncores=NCORES, trace=False):
    plan = _plan(q_src, v_src, feat, src, dst, ncores)
    trivial_affine = bool(
        np.all(np.asarray(ln_gamma, np.float32) == 1.0)
        and np.all(np.asarray(ln_beta, np.float32) == 0.0))
    nc = _get_nc(plan, ncores, trivial_affine)
    in_maps = _make_inmaps(
        plan, (Wk, bk, Wskip, bskip, Wgate, bgate, ln_gamma, ln_beta, prelu_a),
        ncores)
    res = run_bass_kernel_spmd(nc, in_maps, core_ids=list(range(ncores)),
                               trace=trace)
    n, npc, ngrp = plan["n"], plan["npc"], plan["ngrp"]
    out = np.empty((n, HD), np.float32)
    for c in range(ncores):
        r = np.asarray(res.results[c]["out"]).astype(np.float32)
        # [P, ngrp, D, H] -> [ngrp, P, H, D] -> [grid, HD]
        arr = r.reshape(P, ngrp, D, H).transpose(1, 0, 3, 2).reshape(-1, HD)
        out[c * npc + plan["cores"][c]["perm"]] = \
            arr[plan["ndum"]:plan["ndum"] + npc]
    return out, res, plan, in_maps, nc


def kernel(**inputs):
    out, _, _, _, _ = run(**inputs)
    return out
